# revision 40
# baseline (speedup 1.0000x reference)
"""AssociativeEmbeddingLoss on 8 TRN2 NeuronCores (~35us/exec).

Reference, per image b (C=1, G=128 boxes):
    tl[g] = pred[b, 0, ty[g], tx[g]],  br[g] = target[b, 0, by[g], bx[g]]
    me = (tl + br) / 2
    pull_b = sum((tl-br)^2) / (2N)
    push_b = sum_{i != j} relu(1 - |me_i - me_j|) / (N*(N-1))
    out = (0.25 * sum_b pull_b, 0.25 * sum_b push_b)

Data-parallel over batch, 8 images per core. On this runtime the SWDGE
Q7 ucode generates descriptors at ~8.6ns each and INDIRECT1D takes one
offset per PARTITION (max 128 scattered reads/call, serial across
calls), so the 16-call element-gather floor is ~26us. `dma_gather`
(InstDMAGatherAnt) instead takes any number of int16 chunk indices per
call, and calls on different SWDGE queues generate concurrently, at
the price of a one-time ~9.7us gpsimd `mlp` library reload.

Gather: FOUR dma_gather calls (2 images each) on queues 0..3 fetch the
256-byte chunk holding each box-corner pixel (chunk id = flat//64 into
a per-call [16384, 64] f32 view; idx block wrapped [16, n/16] and
replicated to all 8 Q7-core partition groups). While the reload runs
(DVE is idle), 16 tensor_scalar ops build 0.5-scaled one-hot masks
from tiny host-sent rem columns (match metadata only). Selection per
half (4 images) is one DVE multiply [128, 512] + one 3D tensor_reduce
-> col-layout 0.5*values [128 boxes, tl|br x 4].

Pairwise term per half: me columns (plus a constant-1 dummy column)
are PE-transposed once to [5, 128]; a per-partition scalar multiply
(+1,-1,-1,-1,-1) turns that into the fused matmul lhsT [ones; -me],
and four [128,1]->[1,128] PE transposes + one DVE copy build the flat
me row as row 0 of a [5, 512] rhs whose rows 1..4 are a block
indicator const. ONE K=5 bf16 matmul then yields
D[i, 128b+j] = me_b[j] - me_b[i] in PSUM. D is antisymmetric per
block, so sum min(|D|,1) = 2 * sum clamp(D,0,1): one fused DVE
scalar_tensor_tensor (max 0 / min ones) per half, whose accumulator
sums (tensor_scalar's accum reduces with op1 on HW). pull comes from
the 0.5-scaled cols (host multiplies by 4). Each core DMAs a [128, 4]
partial; the host does the final (all-reduce-equivalent) combine.

Measured budget: ~7.0 launch prologue + ~9.7 library reload + ~5.0
descgen (4-queue floor) + ~3 DMA wave + ~2.9 selection + ~3.4 tail +
~0.7 out + ~3.5 teardown.
"""

import numpy as np

import concourse.bacc as bacc
import concourse.bass as bass
import concourse.mybir as mybir
import concourse.tile as tile
from concourse.bass_utils import run_bass_kernel_spmd

B, C, H, W = 64, 1, 512, 512
G = 128                 # boxes per image; N = G*C = 128
N = G * C
NCORES = 8
BP = B // NCORES        # images per core
HB = BP // 2            # images per half (4)
IMPIX = H * W           # pixels per image
ES = 64                 # f32 elements per gathered chunk (256 B)
NCH = 2 * 2 * IMPIX // ES    # chunk rows per per-call data tensor (16384)
NCALL = 4               # dma_gather calls (2 images each)
NI = 2 * 2 * G          # tokens per call (512: tl/br x 2 images x 128)
CBC = NI // G           # chunk blocks per call (4)
CB = 2 * CBC            # chunk blocks per half (8)
HALF = HB * G           # pairwise-matrix columns per half (512)
PULL_W, PUSH_W = 0.25, 0.25

F32 = mybir.dt.float32
BF16 = mybir.dt.bfloat16
I16 = mybir.dt.int16
ALU = mybir.AluOpType
AX = mybir.AxisListType

# packed f32 consts tensor [G, CW]:
#   cols 0..G           iota row (0..127)
#   cols G..G+16        rem columns (one per (half, block))
#   cols G+16..G+16+G/2 identity, bf16 packed in f32 words
#   remaining B_W/2     blockind|onesrow, bf16 packed (partitions 0..4 / 0)
B_BLK = 0
B_ONES = HALF
B_W = HALF + G
C_IOTA = 0
C_REM = G
C_ID = G + 16
C_AXS = C_ID + G // 2
C_SC5 = C_AXS + B_W // 2
CW = C_SC5 + 1


def _build_nc():
    nc = bacc.Bacc(
        "TRN2",
        target_bir_lowering=False,
        debug=False,
        enable_asserts=False,
        num_devices=1,
        num_swdge_queues=4,
    )
    datas = [nc.dram_tensor(f"data{k}", [NCH, ES], F32, kind="ExternalInput")
             for k in range(NCALL)]
    idxs = nc.dram_tensor("idxs", [G, NCALL * (NI // 16)], I16, kind="ExternalInput")
    datas = datas  # noqa
    consts = nc.dram_tensor("consts", [G, CW], F32, kind="ExternalInput")
    rhs5c = nc.dram_tensor("rhs5c", [HB + 1, HALF], BF16, kind="ExternalInput")
    out = nc.dram_tensor("out", [G, 4], F32, kind="ExternalOutput")

    with tile.TileContext(nc) as tc:
        _kernel_body(nc, tc, datas, idxs, consts, rhs5c, out)
    nc.compile()
    return nc


def _kernel_body(nc, tc, datas, idxs, consts, rhs5c, out):
    with (
        tc.tile_pool(name="sb", bufs=1) as sb,
        tc.tile_pool(name="ps", bufs=1, space="PSUM") as ps,
        tc.tile_pool(name="pst", bufs=1, space="PSUM") as pst,
        tc.tile_pool(name="pstr", bufs=1, space="PSUM") as pstr,
    ):
        # ---- input DMAs (idx first; mask/auxb via the Act HWDGE queue) ----
        ix = sb.tile([G, NCALL * (NI // 16)], I16, tag="ix")
        nc.sync.dma_start(out=ix[:], in_=idxs.ap())
        ct = sb.tile([G, CW], F32, tag="ct")
        nc.sync.dma_start(out=ct[:], in_=consts.ap())
        ctb = ct[:].bitcast(BF16)
        iota = ct[:, C_IOTA : C_IOTA + G]
        ident = ctb[:, 2 * C_ID : 2 * C_ID + G]
        sc5 = ct[0 : HB + 1, C_SC5 : C_SC5 + 1]
        rhs5 = [
            sb.tile([HB + 1, HALF], BF16, name="rhs5A", tag="rhs5A"),
            sb.tile([HB + 1, HALF], BF16, name="rhs5B", tag="rhs5B"),
        ]
        nc.sync.dma_start(out=rhs5[0][:], in_=rhs5c.ap())
        nc.sync.dma_start(out=rhs5[1][:], in_=rhs5c.ap())
        mkb = sb.tile([G, 2 * CB * ES], F32, tag="mkb")

        res = sb.tile([G, 4], F32, tag="res")
        nc.vector.memset(res[:], 0.0)
        onesh = sb.tile([G, HALF], F32, tag="onesh")
        nc.vector.memset(onesh[:], 1.0)

        # ---- four chunk gathers on SWDGE queues 0..3 ----
        ch = [
            sb.tile([G, CB * ES], F32, name="chA", tag="chA"),
            sb.tile([G, CB * ES], F32, name="chB", tag="chB"),
        ]
        for k, data in enumerate(datas):
            h, j = divmod(k, 2)
            dst = ch[h][:, j * CBC * ES : (j + 1) * CBC * ES]
            nc.gpsimd.dma_gather(
                dst.rearrange("p (c e) -> p c e", c=CBC, e=ES),
                data.ap(), ix[:, k * (NI // 16) : (k + 1) * (NI // 16)],
                NI, NI, ES, queue_num=k,
            )

        # Build the 0.5-scaled one-hot selection masks ON-CHIP from the tiny
        # rem columns: DVE is idle during the ~10us gpsimd library reload, and
        # this removes the 0.5MB mask DMA whose transfer contends with the
        # reload's ucode fetch.
        for hh in range(2):
            for c in range(CB):
                nc.vector.tensor_scalar(
                    out=mkb[:, (hh * CB + c) * ES : (hh * CB + c + 1) * ES],
                    in0=iota[:, 0:ES],
                    scalar1=ct[:, C_REM + hh * CB + c : C_REM + hh * CB + c + 1],
                    scalar2=0.5, op0=ALU.is_equal, op1=ALU.mult,
                )

        col = [None, None]
        rowS = [None, None]
        neg = [None, None]
        psT = [None, None]
        rowP = [None, None]

        def select_half(h):
            # col[g, c] = 0.5 * ch[g, c, rem]: masked multiply + reduce.
            # The multiply is split per gather call: the SWDGE queues drain
            # sequentially, so call 2h's chunks land ~0.8us before call
            # 2h+1's -- the first half-multiply runs before that gate.
            prod = sb.tile([G, CB * ES], F32, name=f"prod{h}", tag=f"prod{h}")
            for j in range(2):
                cs = slice(j * CBC * ES, (j + 1) * CBC * ES)
                nc.vector.tensor_tensor(
                    out=prod[:, cs], in0=ch[h][:, cs],
                    in1=mkb[:, h * CB * ES + j * CBC * ES :
                            h * CB * ES + (j + 1) * CBC * ES], op=ALU.mult,
                )
            col[h] = sb.tile([G, CB], F32, name=f"col{h}", tag=f"col{h}")
            nc.vector.tensor_reduce(
                out=col[h][:],
                in_=prod[:].rearrange("p (c e) -> p c e", c=CB, e=ES),
                axis=AX.X, op=ALU.add,
            )

        me2hx = [
            sb.tile([G, HB + 1], BF16, name="me2hxA", tag="me2hxA"),
            sb.tile([G, HB + 1], BF16, name="me2hxB", tag="me2hxB"),
        ]
        nc.vector.memset(me2hx[0][:, 0:1], 1.0)
        nc.vector.memset(me2hx[1][:, 0:1], 1.0)

        def chain_half(h):
            # column order per half: [tl0 br0 tl1 br1 tl2 br2 tl3 br3]
            # me (=0.5tl+0.5br; mask pre-scales by 0.5) in bf16, col layout.
            # me2hx col 0 is a constant-1 dummy: after the transpose it gives
            # the +1 weights row of the fused K=5 matmul.
            c3 = col[h][:].rearrange("p (q t) -> p q t", q=HB, t=2)
            nc.vector.scalar_tensor_tensor(
                out=me2hx[h][:, 1 : HB + 1], in0=c3[:, :, 0:1], scalar=1.0,
                in1=c3[:, :, 1:2], op0=ALU.bypass, op1=ALU.add,
            )
            psT[h] = pst.tile([HB + 1, G], BF16, name=f"psT{h}", tag=f"psT{h}")
            nc.tensor.transpose(out=psT[h][:], in_=me2hx[h][:], identity=ident)
            # negx rows: [ones; -me0..3] via per-partition scalars (+1,-1,..)
            neg[h] = sb.tile([HB + 1, G], BF16, name=f"neg{h}", tag=f"neg{h}")
            nc.vector.tensor_scalar(
                out=neg[h][:], in0=psT[h][:], scalar1=sc5, scalar2=None,
                op0=ALU.mult,
            )
            # flat me row [1, 512] via four [128,1]->[1,128] PE transposes
            # (a flatten DMA costs a ~1.7us round trip; PE is idle here)
            rowP[h] = pstr.tile([1, HALF], BF16, name=f"rowP{h}", tag=f"rowP{h}")
            for q in range(HB):
                nc.tensor.transpose(
                    out=rowP[h][0:1, q * G : (q + 1) * G],
                    in_=me2hx[h][:, 1 + q : 2 + q], identity=ident,
                )
            nc.vector.tensor_copy(out=rhs5[h][0:1, :], in_=rowP[h][:])

        scr = sb.tile([G, HALF], F32, tag="scr")
        Dps = [None, None]

        def mm_half(h):
            Dps[h] = ps.tile([G, HALF], F32, name=f"D{h}", tag=f"D{h}")
            nc.tensor.matmul(
                out=Dps[h][:], lhsT=neg[h][:], rhs=rhs5[h][:],
                start=True, stop=True,
            )

        def minacc_half(h):
            # D antisymmetric per block: sum min(|D|,1) = 2*sum clamp(D,0,1).
            # scalar_tensor_tensor: its accumulator sums (tensor_scalar's
            # accum_out reduces with op1 on HW and cannot sum a clamp).
            nc.vector.scalar_tensor_tensor(
                out=scr[:], in0=Dps[h][:], scalar=0.0, in1=onesh[:],
                op0=ALU.max, op1=ALU.min, accum_out=res[:, h : h + 1],
            )

        def pull_half(h):
            # dsub = 0.5*(tl-br) per image (host multiplies pull by 4)
            dsub = sb.tile([G, HB], F32, name=f"dsub{h}", tag=f"dsub{h}")
            c3 = col[h][:].rearrange("p (q t) -> p q t", q=HB, t=2)
            nc.vector.tensor_tensor(
                out=dsub[:], in0=c3[:, :, 0:1], in1=c3[:, :, 1:2],
                op=ALU.subtract,
            )
            sq = sb.tile([G, HB], F32, name=f"sq{h}", tag=f"sq{h}")
            nc.vector.scalar_tensor_tensor(
                out=sq[:], in0=dsub[:], scalar=1.0, in1=dsub[:],
                op0=ALU.mult, op1=ALU.mult, accum_out=res[:, 2 + h : 3 + h],
            )

        select_half(0)
        chain_half(0)
        mm_half(0)
        select_half(1)
        chain_half(1)
        minacc_half(0)
        pull_half(0)
        mm_half(1)
        minacc_half(1)
        pull_half(1)

        nc.sync.dma_start(out=out.ap(), in_=res[:])


_NC_CACHE = None


def _get_nc():
    global _NC_CACHE
    if _NC_CACHE is None:
        _NC_CACHE = _build_nc()
    return _NC_CACHE


def _wrap_idxs(chunk_ids):
    """[512] -> [128, 32]: token i at [i%16, i//16], replicated x8."""
    w = np.zeros((16, NI // 16), dtype=np.int16)
    w[np.arange(NI) % 16, np.arange(NI) // 16] = chunk_ids
    return np.tile(w, (8, 1))


def _constpack():
    """bf16 consts packed as f32 words: ident [G, G//2], axs [G, B_W//2]."""
    import ml_dtypes
    ab = np.eye(G, dtype=np.float32).astype(ml_dtypes.bfloat16)
    axs = np.zeros((G, B_W), dtype=np.float32)
    for q in range(HB):
        axs[q, B_BLK + q * G : B_BLK + (q + 1) * G] = 1.0
    axs[0, B_ONES : B_ONES + G] = 1.0
    axs = axs.astype(ml_dtypes.bfloat16)
    return (np.ascontiguousarray(ab).view(np.uint16).reshape(G, G),
            np.ascontiguousarray(axs).view(np.uint16).reshape(G, B_W))


def make_in_maps(pred, target, match):
    pred = np.asarray(pred, dtype=np.float32).reshape(B, IMPIX)
    target = np.asarray(target, dtype=np.float32).reshape(B, IMPIX)
    match = np.asarray(match)
    ty = match[:, :, 0, 0].astype(np.int64)
    tx = match[:, :, 0, 1].astype(np.int64)
    by = match[:, :, 1, 0].astype(np.int64)
    bx = match[:, :, 1, 1].astype(np.int64)
    tlf = (ty * W + tx).astype(np.int32)               # [B, G] flat in-image
    brf = (by * W + bx).astype(np.int32)
    ident_u16, axs_u16 = _constpack()
    in_maps = []
    for k in range(NCORES):
        im = {}
        ix_all = np.empty((G, NCALL * (NI // 16)), dtype=np.int16)
        consts = np.zeros((G, CW), dtype=np.float32)
        consts[:, C_IOTA : C_IOTA + G] = np.arange(G, dtype=np.float32)[None, :]
        consts[0, C_SC5] = 1.0
        consts[1 : HB + 1, C_SC5] = -1.0
        cu16 = consts.view(np.uint16).reshape(G, -1)
        for kcall in range(NCALL):   # call covers core-images 2k, 2k+1
            imgs = [k * BP + 2 * kcall + qq for qq in range(2)]
            dat = np.empty((NCH, ES), dtype=np.float32)
            dat[: NCH // 2] = pred[imgs].reshape(-1, ES)
            dat[NCH // 2 :] = target[imgs].reshape(-1, ES)
            im[f"data{kcall}"] = dat
            h, j = divmod(kcall, 2)
            c_ids = np.empty((CBC, G), dtype=np.int64)
            rem = np.empty((CBC, G), dtype=np.int64)
            for qq in range(2):      # image index within call; tl/br paired
                fl = tlf[imgs[qq]]
                c_ids[2 * qq] = qq * (IMPIX // ES) + fl // ES
                rem[2 * qq] = fl % ES
                fb = brf[imgs[qq]]
                c_ids[2 * qq + 1] = NCH // 2 + qq * (IMPIX // ES) + fb // ES
                rem[2 * qq + 1] = fb % ES
            ix_all[:, kcall * (NI // 16) : (kcall + 1) * (NI // 16)] = (
                _wrap_idxs(c_ids.reshape(-1).astype(np.int16)))
            # rem columns: one per (half, block); block = j*CBC + c
            for c in range(CBC):
                consts[:, C_REM + h * CB + j * CBC + c] = rem[c]
        cu16[:, 2 * C_ID : 2 * C_ID + G] = ident_u16
        cu16[:, 2 * C_AXS : 2 * C_AXS + B_W] = axs_u16
        im["idxs"] = ix_all
        im["consts"] = consts
        im["rhs5c"] = None  # filled in kernel()
        in_maps.append(im)
    return in_maps


def _to_bf16(x):
    import ml_dtypes
    return x.astype(ml_dtypes.bfloat16)


def _rhs5c():
    import ml_dtypes
    r = np.zeros((HB + 1, HALF), dtype=np.float32)
    for q in range(HB):
        r[1 + q, q * G : (q + 1) * G] = 1.0
    return r.astype(ml_dtypes.bfloat16)


def kernel(pred, target, match, _trace=False):
    nc = _get_nc()
    in_maps = make_in_maps(pred, target, match)
    r5 = _rhs5c()
    for im in in_maps:
        im["rhs5c"] = r5
    res = run_bass_kernel_spmd(nc, in_maps, core_ids=list(range(NCORES)), trace=_trace)
    minsum = 0.0
    pullsum = 0.0
    for r in res.results:
        o = r["out"].astype(np.float64)
        minsum += 2.0 * (o[:, 0].sum() + o[:, 1].sum())
        pullsum += 4.0 * (o[:, 2].sum() + o[:, 3].sum())
    pull = PULL_W * pullsum / (2.0 * N)
    push = PUSH_W * (B * N * (N - 1) - minsum) / (N * (N - 1))
    out = (np.float32(pull), np.float32(push))
    if _trace:
        return out, res
    return out


# revision 41
# speedup vs baseline: 1.0129x; 1.0129x over previous
"""AssociativeEmbeddingLoss on 8 TRN2 NeuronCores (~35us/exec).

Reference, per image b (C=1, G=128 boxes):
    tl[g] = pred[b, 0, ty[g], tx[g]],  br[g] = target[b, 0, by[g], bx[g]]
    me = (tl + br) / 2
    pull_b = sum((tl-br)^2) / (2N)
    push_b = sum_{i != j} relu(1 - |me_i - me_j|) / (N*(N-1))
    out = (0.25 * sum_b pull_b, 0.25 * sum_b push_b)

Data-parallel over batch, 8 images per core. On this runtime the SWDGE
Q7 ucode generates descriptors at ~8.6ns each and INDIRECT1D takes one
offset per PARTITION (max 128 scattered reads/call, serial across
calls), so the 16-call element-gather floor is ~26us. `dma_gather`
(InstDMAGatherAnt) instead takes any number of int16 chunk indices per
call, and calls on different SWDGE queues generate concurrently, at
the price of a one-time ~9.7us gpsimd `mlp` library reload.

Gather: FOUR dma_gather calls (2 images each) on queues 0..3 fetch the
256-byte chunk holding each box-corner pixel (chunk id = flat//64 into
a per-call [16384, 64] f32 view; idx block wrapped [16, n/16] and
replicated to all 8 Q7-core partition groups). While the reload runs
(DVE is idle), 16 tensor_scalar ops build 0.5-scaled one-hot masks
from tiny host-sent rem columns (match metadata only). Selection per
half (4 images) is one DVE multiply [128, 512] + one 3D tensor_reduce
-> col-layout 0.5*values [128 boxes, tl|br x 4].

Pairwise term per half: me columns (plus a constant-1 dummy column)
are PE-transposed once to [5, 128]; a per-partition scalar multiply
(+1,-1,-1,-1,-1) turns that into the fused matmul lhsT [ones; -me],
and four [128,1]->[1,128] PE transposes + one DVE copy build the flat
me row as row 0 of a [5, 512] rhs whose rows 1..4 are a block
indicator const. ONE K=5 bf16 matmul then yields
D[i, 128b+j] = me_b[j] - me_b[i] in PSUM. D is antisymmetric per
block, so sum min(|D|,1) = 2 * sum clamp(D,0,1): one fused DVE
scalar_tensor_tensor (max 0 / min ones) per half, whose accumulator
sums (tensor_scalar's accum reduces with op1 on HW). pull comes from
the 0.5-scaled cols (host multiplies by 4). Each core DMAs a [128, 4]
partial; the host does the final (all-reduce-equivalent) combine.

Measured budget: ~7.0 launch prologue + ~9.7 library reload + ~5.0
descgen (4-queue floor) + ~3 DMA wave + ~2.9 selection + ~3.4 tail +
~0.7 out + ~3.5 teardown.
"""

import numpy as np

import concourse.bacc as bacc
import concourse.bass as bass
import concourse.mybir as mybir
import concourse.tile as tile
from concourse.bass_utils import run_bass_kernel_spmd

B, C, H, W = 64, 1, 512, 512
G = 128                 # boxes per image; N = G*C = 128
N = G * C
NCORES = 8
BP = B // NCORES        # images per core
HB = BP // 2            # images per half (4)
IMPIX = H * W           # pixels per image
ES = 64                 # f32 elements per gathered chunk (256 B)
NCH = 2 * 2 * IMPIX // ES    # chunk rows per per-call data tensor (16384)
NCALL = 4               # dma_gather calls (2 images each)
NI = 2 * 2 * G          # tokens per call (512: tl/br x 2 images x 128)
CBC = NI // G           # chunk blocks per call (4)
CB = 2 * CBC            # chunk blocks per half (8)
HALF = HB * G           # pairwise-matrix columns per half (512)
PULL_W, PUSH_W = 0.25, 0.25

F32 = mybir.dt.float32
BF16 = mybir.dt.bfloat16
I16 = mybir.dt.int16
ALU = mybir.AluOpType
AX = mybir.AxisListType

# packed f32 consts tensor [G, CW]:
#   cols 0..G           iota row (0..127)
#   cols G..G+16        rem columns (one per (half, block))
#   cols G+16..G+16+G/2 identity, bf16 packed in f32 words
#   remaining B_W/2     blockind|onesrow, bf16 packed (partitions 0..4 / 0)
B_BLK = 0
B_ONES = HALF
B_W = HALF + G
C_IOTA = 0
C_REM = G
C_ID = G + 16
C_AXS = C_ID + G // 2
C_SC5 = C_AXS + B_W // 2
CW = C_SC5 + 1


def _build_nc():
    nc = bacc.Bacc(
        "TRN2",
        target_bir_lowering=False,
        debug=False,
        enable_asserts=False,
        num_devices=1,
        num_swdge_queues=4,
    )
    datas = [nc.dram_tensor(f"data{k}", [NCH, ES], F32, kind="ExternalInput")
             for k in range(NCALL)]
    idxs = nc.dram_tensor("idxs", [G, NCALL * (NI // 16)], I16, kind="ExternalInput")
    datas = datas  # noqa
    consts = nc.dram_tensor("consts", [G, CW], F32, kind="ExternalInput")
    rhs5c = nc.dram_tensor("rhs5c", [HB + 1, HALF], BF16, kind="ExternalInput")
    out = nc.dram_tensor("out", [G, 4], F32, kind="ExternalOutput")

    with tile.TileContext(nc) as tc:
        _kernel_body(nc, tc, datas, idxs, consts, rhs5c, out)
    nc.compile()
    return nc


def _kernel_body(nc, tc, datas, idxs, consts, rhs5c, out):
    with (
        tc.tile_pool(name="sb", bufs=1) as sb,
        tc.tile_pool(name="ps", bufs=1, space="PSUM") as ps,
        tc.tile_pool(name="pst", bufs=1, space="PSUM") as pst,
        tc.tile_pool(name="pstr", bufs=1, space="PSUM") as pstr,
    ):
        # ---- input DMAs (idx first; mask/auxb via the Act HWDGE queue) ----
        ix = sb.tile([G, NCALL * (NI // 16)], I16, tag="ix")
        nc.sync.dma_start(out=ix[:], in_=idxs.ap())
        ct = sb.tile([G, CW], F32, tag="ct")
        nc.sync.dma_start(out=ct[:], in_=consts.ap())
        ctb = ct[:].bitcast(BF16)
        iota = ct[:, C_IOTA : C_IOTA + G]
        ident = ctb[:, 2 * C_ID : 2 * C_ID + G]
        sc5 = ct[0 : HB + 1, C_SC5 : C_SC5 + 1]
        rhs5 = [
            sb.tile([HB + 1, HALF], BF16, name="rhs5A", tag="rhs5A"),
            sb.tile([HB + 1, HALF], BF16, name="rhs5B", tag="rhs5B"),
        ]
        nc.sync.dma_start(out=rhs5[0][:], in_=rhs5c.ap())
        nc.sync.dma_start(out=rhs5[1][:], in_=rhs5c.ap())
        mkb = sb.tile([G, 2 * CB * ES], F32, tag="mkb")

        res = sb.tile([G, 4], F32, tag="res")
        nc.vector.memset(res[:], 0.0)
        onesh = sb.tile([G, HALF], F32, tag="onesh")
        nc.vector.memset(onesh[:], 1.0)

        # ---- four chunk gathers on SWDGE queues 0..3 ----
        ch = [
            sb.tile([G, CB * ES], F32, name="chA", tag="chA"),
            sb.tile([G, CB * ES], F32, name="chB", tag="chB"),
        ]
        for k, data in enumerate(datas):
            h, j = divmod(k, 2)
            dst = ch[h][:, j * CBC * ES : (j + 1) * CBC * ES]
            nc.gpsimd.dma_gather(
                dst.rearrange("p (c e) -> p c e", c=CBC, e=ES),
                data.ap(), ix[:, k * (NI // 16) : (k + 1) * (NI // 16)],
                NI, NI, ES, queue_num=k,
            )

        # Build the 0.5-scaled one-hot selection masks ON-CHIP from the tiny
        # rem columns: DVE is idle during the ~10us gpsimd library reload, and
        # this removes the 0.5MB mask DMA whose transfer contends with the
        # reload's ucode fetch.
        for hh in range(2):
            for c in range(CB):
                nc.vector.tensor_scalar(
                    out=mkb[:, (hh * CB + c) * ES : (hh * CB + c + 1) * ES],
                    in0=iota[:, 0:ES],
                    scalar1=ct[:, C_REM + hh * CB + c : C_REM + hh * CB + c + 1],
                    scalar2=0.5, op0=ALU.is_equal, op1=ALU.mult,
                )

        col = [None, None]
        rowS = [None, None]
        neg = [None, None]
        psT = [None, None]
        rowP = [None, None]

        def select_half(h):
            # col[g, c] = 0.5 * ch[g, c, rem]: masked multiply + reduce
            prod = sb.tile([G, CB * ES], F32, name=f"prod{h}", tag=f"prod{h}")
            nc.vector.tensor_tensor(
                out=prod[:], in0=ch[h][:],
                in1=mkb[:, h * CB * ES : (h + 1) * CB * ES], op=ALU.mult,
            )
            col[h] = sb.tile([G, CB], F32, name=f"col{h}", tag=f"col{h}")
            nc.vector.tensor_reduce(
                out=col[h][:],
                in_=prod[:].rearrange("p (c e) -> p c e", c=CB, e=ES),
                axis=AX.X, op=ALU.add,
            )

        me2hx = [
            sb.tile([G, HB + 1], BF16, name="me2hxA", tag="me2hxA"),
            sb.tile([G, HB + 1], BF16, name="me2hxB", tag="me2hxB"),
        ]
        nc.vector.memset(me2hx[0][:, 0:1], 1.0)
        nc.vector.memset(me2hx[1][:, 0:1], 1.0)

        def chain_half(h):
            # column order per half: [tl0 br0 tl1 br1 tl2 br2 tl3 br3]
            # me (=0.5tl+0.5br; mask pre-scales by 0.5) in bf16, col layout.
            # me2hx col 0 is a constant-1 dummy: after the transpose it gives
            # the +1 weights row of the fused K=5 matmul.
            c3 = col[h][:].rearrange("p (q t) -> p q t", q=HB, t=2)
            nc.vector.scalar_tensor_tensor(
                out=me2hx[h][:, 1 : HB + 1], in0=c3[:, :, 0:1], scalar=1.0,
                in1=c3[:, :, 1:2], op0=ALU.bypass, op1=ALU.add,
            )
            psT[h] = pst.tile([HB + 1, G], BF16, name=f"psT{h}", tag=f"psT{h}")
            nc.tensor.transpose(out=psT[h][:], in_=me2hx[h][:], identity=ident)
            # negx rows: [ones; -me0..3] via per-partition scalars (+1,-1,..)
            neg[h] = sb.tile([HB + 1, G], BF16, name=f"neg{h}", tag=f"neg{h}")
            nc.vector.tensor_scalar(
                out=neg[h][:], in0=psT[h][:], scalar1=sc5, scalar2=None,
                op0=ALU.mult,
            )
            # flat me row [1, 512] via four [128,1]->[1,128] PE transposes
            # (a flatten DMA costs a ~1.7us round trip; PE is idle here)
            rowP[h] = pstr.tile([1, HALF], BF16, name=f"rowP{h}", tag=f"rowP{h}")
            for q in range(HB):
                nc.tensor.transpose(
                    out=rowP[h][0:1, q * G : (q + 1) * G],
                    in_=me2hx[h][:, 1 + q : 2 + q], identity=ident,
                )
            nc.vector.tensor_copy(out=rhs5[h][0:1, :], in_=rowP[h][:])

        scr = sb.tile([G, HALF], F32, tag="scr")
        Dps = [None, None]

        def mm_half(h):
            Dps[h] = ps.tile([G, HALF], F32, name=f"D{h}", tag=f"D{h}")
            nc.tensor.matmul(
                out=Dps[h][:], lhsT=neg[h][:], rhs=rhs5[h][:],
                start=True, stop=True,
            )

        def minacc_half(h):
            # D antisymmetric per block: sum min(|D|,1) = 2*sum clamp(D,0,1).
            # scalar_tensor_tensor: its accumulator sums (tensor_scalar's
            # accum_out reduces with op1 on HW and cannot sum a clamp).
            nc.vector.scalar_tensor_tensor(
                out=scr[:], in0=Dps[h][:], scalar=0.0, in1=onesh[:],
                op0=ALU.max, op1=ALU.min, accum_out=res[:, h : h + 1],
            )

        def pull_half(h):
            # dsub = 0.5*(tl-br) per image (host multiplies pull by 4)
            dsub = sb.tile([G, HB], F32, name=f"dsub{h}", tag=f"dsub{h}")
            c3 = col[h][:].rearrange("p (q t) -> p q t", q=HB, t=2)
            nc.vector.tensor_tensor(
                out=dsub[:], in0=c3[:, :, 0:1], in1=c3[:, :, 1:2],
                op=ALU.subtract,
            )
            sq = sb.tile([G, HB], F32, name=f"sq{h}", tag=f"sq{h}")
            nc.vector.scalar_tensor_tensor(
                out=sq[:], in0=dsub[:], scalar=1.0, in1=dsub[:],
                op0=ALU.mult, op1=ALU.mult, accum_out=res[:, 2 + h : 3 + h],
            )

        select_half(0)
        chain_half(0)
        mm_half(0)
        select_half(1)
        chain_half(1)
        minacc_half(0)
        pull_half(0)
        mm_half(1)
        minacc_half(1)
        pull_half(1)

        nc.sync.dma_start(out=out.ap(), in_=res[:])


_NC_CACHE = None


def _get_nc():
    global _NC_CACHE
    if _NC_CACHE is None:
        _NC_CACHE = _build_nc()
    return _NC_CACHE


def _wrap_idxs(chunk_ids):
    """[512] -> [128, 32]: token i at [i%16, i//16], replicated x8."""
    w = np.zeros((16, NI // 16), dtype=np.int16)
    w[np.arange(NI) % 16, np.arange(NI) // 16] = chunk_ids
    return np.tile(w, (8, 1))


def _constpack():
    """bf16 consts packed as f32 words: ident [G, G//2], axs [G, B_W//2]."""
    import ml_dtypes
    ab = np.eye(G, dtype=np.float32).astype(ml_dtypes.bfloat16)
    axs = np.zeros((G, B_W), dtype=np.float32)
    for q in range(HB):
        axs[q, B_BLK + q * G : B_BLK + (q + 1) * G] = 1.0
    axs[0, B_ONES : B_ONES + G] = 1.0
    axs = axs.astype(ml_dtypes.bfloat16)
    return (np.ascontiguousarray(ab).view(np.uint16).reshape(G, G),
            np.ascontiguousarray(axs).view(np.uint16).reshape(G, B_W))


def make_in_maps(pred, target, match):
    pred = np.asarray(pred, dtype=np.float32).reshape(B, IMPIX)
    target = np.asarray(target, dtype=np.float32).reshape(B, IMPIX)
    match = np.asarray(match)
    ty = match[:, :, 0, 0].astype(np.int64)
    tx = match[:, :, 0, 1].astype(np.int64)
    by = match[:, :, 1, 0].astype(np.int64)
    bx = match[:, :, 1, 1].astype(np.int64)
    tlf = (ty * W + tx).astype(np.int32)               # [B, G] flat in-image
    brf = (by * W + bx).astype(np.int32)
    ident_u16, axs_u16 = _constpack()
    in_maps = []
    for k in range(NCORES):
        im = {}
        ix_all = np.empty((G, NCALL * (NI // 16)), dtype=np.int16)
        consts = np.zeros((G, CW), dtype=np.float32)
        consts[:, C_IOTA : C_IOTA + G] = np.arange(G, dtype=np.float32)[None, :]
        consts[0, C_SC5] = 1.0
        consts[1 : HB + 1, C_SC5] = -1.0
        cu16 = consts.view(np.uint16).reshape(G, -1)
        for kcall in range(NCALL):   # call covers core-images 2k, 2k+1
            imgs = [k * BP + 2 * kcall + qq for qq in range(2)]
            dat = np.empty((NCH, ES), dtype=np.float32)
            dat[: NCH // 2] = pred[imgs].reshape(-1, ES)
            dat[NCH // 2 :] = target[imgs].reshape(-1, ES)
            im[f"data{kcall}"] = dat
            h, j = divmod(kcall, 2)
            c_ids = np.empty((CBC, G), dtype=np.int64)
            rem = np.empty((CBC, G), dtype=np.int64)
            for qq in range(2):      # image index within call; tl/br paired
                fl = tlf[imgs[qq]]
                c_ids[2 * qq] = qq * (IMPIX // ES) + fl // ES
                rem[2 * qq] = fl % ES
                fb = brf[imgs[qq]]
                c_ids[2 * qq + 1] = NCH // 2 + qq * (IMPIX // ES) + fb // ES
                rem[2 * qq + 1] = fb % ES
            ix_all[:, kcall * (NI // 16) : (kcall + 1) * (NI // 16)] = (
                _wrap_idxs(c_ids.reshape(-1).astype(np.int16)))
            # rem columns: one per (half, block); block = j*CBC + c
            for c in range(CBC):
                consts[:, C_REM + h * CB + j * CBC + c] = rem[c]
        cu16[:, 2 * C_ID : 2 * C_ID + G] = ident_u16
        cu16[:, 2 * C_AXS : 2 * C_AXS + B_W] = axs_u16
        im["idxs"] = ix_all
        im["consts"] = consts
        im["rhs5c"] = None  # filled in kernel()
        in_maps.append(im)
    return in_maps


def _to_bf16(x):
    import ml_dtypes
    return x.astype(ml_dtypes.bfloat16)


def _rhs5c():
    import ml_dtypes
    r = np.zeros((HB + 1, HALF), dtype=np.float32)
    for q in range(HB):
        r[1 + q, q * G : (q + 1) * G] = 1.0
    return r.astype(ml_dtypes.bfloat16)


def kernel(pred, target, match, _trace=False):
    nc = _get_nc()
    in_maps = make_in_maps(pred, target, match)
    r5 = _rhs5c()
    for im in in_maps:
        im["rhs5c"] = r5
    res = run_bass_kernel_spmd(nc, in_maps, core_ids=list(range(NCORES)), trace=_trace)
    minsum = 0.0
    pullsum = 0.0
    for r in res.results:
        o = r["out"].astype(np.float64)
        minsum += 2.0 * (o[:, 0].sum() + o[:, 1].sum())
        pullsum += 4.0 * (o[:, 2].sum() + o[:, 3].sum())
    pull = PULL_W * pullsum / (2.0 * N)
    push = PUSH_W * (B * N * (N - 1) - minsum) / (N * (N - 1))
    out = (np.float32(pull), np.float32(push))
    if _trace:
        return out, res
    return out
